# revision 8
# baseline (speedup 1.0000x reference)
"""Trainium2 Bass kernel: batched controlled-system dynamics (N = 2^20 states).

Strategy:
  - Pure data parallel over 8 NeuronCores: contiguous slices of the batch axis.
  - Per core the workload is one [128, 1024] fp32 SBUF "ribbon" per vector.
  - The friction MLP (1 -> 64 -> 2, tanh + softplus heads) depends only on the
    scalar v2, so it is collapsed host-side into 1D functions of v2:
      kinetic(v2)  = softplus(g0(v2) + b2[0]),  g0 odd for b1 == 0
      stiction(v2) = softplus(g1(v2) + b2[1]),  only needed for |v2| < 0.01
    g0 is approximated as v2 * O(w), O a polynomial in w = affine(v2^2), fit at
    runtime from the actual W1/W2/b1/b2; softplus is evaluated on-chip as
    Ln(1 + Exp(x)); stiction is linearized around 0 (exact to ~1e-4 on the
    +-0.01 window where it is used).
  - Work is split across engines: Vector (fused scalar_tensor_tensor chains +
    Horner), GpSimd (independent tensor_tensor/tensor_scalar ops), Scalar
    (Sin/Square/Sign/Abs + Exp/Ln + Identity-affine; 2 activation-table sets).
"""

import numpy as np

# physical system constants (match the reference)
M1, M2 = 1.0, 1.5
K1, K2 = 2.0, 3.0
C1, C2 = 0.5, 0.8
KARNOPP_DV = 0.01
REF_AMP, REF_OMEGA = 0.5, 0.5

N_CORES = 8
N_TOTAL = 1 << 20
N_CORE = N_TOTAL // N_CORES    # 131072
P = 128
F = N_CORE // P                # 1024

HORNER_BF16 = True
FIT_TOL = 2.5e-3

_compile_cache = {}


def _softplus(x):
    return np.log1p(np.exp(-np.abs(x))) + np.maximum(x, 0.0)


def _fit_friction(W1, b1, W2, b2, vmax):
    """Fit the 1D collapse of the friction MLP (see module docstring)."""
    W1 = W1.astype(np.float64).reshape(-1)      # [H]
    b1 = b1.astype(np.float64).reshape(-1)
    W2 = W2.astype(np.float64)                  # [H, 2]
    b2 = b2.astype(np.float64).reshape(-1)

    def gg(v, col):
        return np.tanh(np.outer(v, W1) + b1) @ W2[:, col]

    umax = vmax * vmax
    su = 2.0 / umax
    bu = -1.0

    M = 4000
    wn = np.cos(np.pi * (np.arange(M) + 0.5) / M)
    u = (wn + 1.0) / 2.0 * umax
    v = np.sqrt(np.maximum(u, 1e-12))
    gp = gg(v, 0)
    gm = gg(-v, 0)
    E = (gp + gm) / 2.0          # even part of g0 (== 0 when b1 == 0)
    O = (gp - gm) / 2.0 / v      # odd part / v, a function of u (hence w)

    exp_bias = b2[0] + float(np.mean(E))

    vchk = np.linspace(KARNOPP_DV * 0.9, vmax, 30000)
    uchk = vchk * vchk
    wchk = uchk * su + bu
    g0chk = (gg(vchk, 0) - gg(-vchk, 0)) / 2.0

    weight = v + 0.02
    import numpy.polynomial.chebyshev as C
    import numpy.polynomial.polynomial as Pp

    best = None
    for deg in range(6, 17):
        cc = C.chebfit(wn, O, deg, w=weight)
        mono = C.cheb2poly(cc)
        err = np.abs(vchk * Pp.polyval(wchk, mono) - g0chk).max()
        best = (mono, err)
        if err < FIT_TOL:
            break
    mono, err = best

    # stiction limit, linearized at 0:  L(v) = softplus(g1(v) + b2[1])
    d = 1e-4
    g1p = (gg(np.array([d]), 1)[0] - gg(np.array([-d]), 1)[0]) / (2 * d)
    g10 = gg(np.array([0.0]), 1)[0] + b2[1]
    L0 = _softplus(g10)
    sig = 1.0 / (1.0 + np.exp(-g10))
    L1 = sig * g1p

    return dict(oc=mono, su=su, bu=bu, exp_bias=exp_bias, L0=L0, L1=L1,
                fit_err=err, deg=len(mono) - 1)


def _build_program(consts):
    """Build the SPMD Bass program (same on all 8 cores)."""
    import concourse.bacc as bacc
    import concourse.mybir as mybir
    from concourse import tile
    from concourse.tile_rust import add_dep_helper

    fp32 = mybir.dt.float32
    bf16 = mybir.dt.bfloat16
    hdt = bf16 if HORNER_BF16 else fp32
    Alu = mybir.AluOpType
    Act = mybir.ActivationFunctionType

    c = {k: float(np.float32(v)) for k, v in consts.items() if np.isscalar(v)}
    oc = [float(np.float32(x)) for x in consts["oc"]]
    deg = len(oc) - 1

    nc = bacc.Bacc()

    # activation float biases lower to [128,1] const SBUF APs; register the
    # ones this program uses (0.0 / 1.0 ship with Bass already)
    def reg_const(val):
        v = float(val)
        if (fp32, v) not in nc.const_aps.aps:
            tsr = nc.alloc_sbuf_tensor(f"constu-f32-{len(nc.const_aps.aps)}", [128, 1], fp32)
            nc.gpsimd.memset(tsr.ap(), v)
            nc.const_aps.aps[(fp32, v)] = tsr.ap()

    neg_pi = float(np.float32(-np.pi))
    for v in (neg_pi, c["exp_bias"], c["L0_d"], -c["L0_d"]):
        reg_const(v)
    nc.all_engine_barrier()

    t_d = nc.dram_tensor("t", [N_CORE], fp32, kind="ExternalInput")
    z_d = nc.dram_tensor("z", [5, N_CORE], fp32, kind="ExternalInput")
    out_d = nc.dram_tensor("out", [5, N_CORE], fp32, kind="ExternalOutput")

    t_r = t_d[:].rearrange("(p f) -> p f", p=P)
    z_r = [z_d[i, :].rearrange("(p f) -> p f", p=P) for i in range(5)]
    o_r = [out_d[i, :].rearrange("(p f) -> p f", p=P) for i in range(5)]

    with tile.TileContext(nc) as tc:
        with tc.tile_pool(name="sb", bufs=1) as pool:
            def tl(tag, dt=fp32):
                return pool.tile([P, F], dt, tag=tag, name=tag)

            T = tl("T"); X1 = tl("X1"); V1 = tl("V1"); X2 = tl("X2")
            V2 = tl("V2"); XC = tl("XC")

            # loads
            nc.sync.dma_start(out=T[:], in_=t_r)
            nc.sync.dma_start(out=X1[:], in_=z_r[0])
            nc.sync.dma_start(out=V1[:], in_=z_r[1])
            nc.sync.dma_start(out=X2[:], in_=z_r[2])
            nc.sync.dma_start(out=V2[:], in_=z_r[3])
            nc.sync.dma_start(out=XC[:], in_=z_r[4])

            # passthrough rows: dx1 = v1, dx2 = v2
            nc.sync.dma_start(out=o_r[0], in_=V1[:])
            nc.sync.dma_start(out=o_r[2], in_=V2[:])

            # ---- ACT phase 1 (table set: trig_and_small) ----
            S = tl("S")      # S = sin(0.5 t - pi) = -sin(0.5 t)
            i_sin = nc.scalar.activation(S[:], T[:], Act.Sin, bias=neg_pi, scale=0.5)
            U = tl("U")      # v2^2
            i_u = nc.scalar.activation(U[:], V2[:], Act.Square)
            SGN = tl("SGN")
            i_sgn = nc.scalar.activation(SGN[:], V2[:], Act.Sign)
            AV = tl("AV")
            i_av = nc.scalar.activation(AV[:], V2[:], Act.Abs)
            # stiction limit (scaled by 1/K2), linear in v2 — Identity is in
            # every table set so these never force a table switch
            LP = tl("LP"); NLP = tl("NLP")
            i_lp = nc.scalar.activation(LP[:], V2[:], Act.Identity,
                                        bias=c["L0_d"], scale=c["L1_d"])
            i_nlp = nc.scalar.activation(NLP[:], V2[:], Act.Identity,
                                         bias=-c["L0_d"], scale=-c["L1_d"])
            V2H = tl("V2H", hdt)
            i_v2h = nc.scalar.activation(V2H[:], V2[:], Act.Copy)
            # keep every phase-1 ACT op after Sin so exactly one trig load +
            # one exp/ln load are inserted (Identity/Square/... are in every
            # set; ordering just stops a third "default" set load)
            for bi in (i_u, i_sgn, i_av, i_lp, i_nlp, i_v2h):
                add_dep_helper(bi.ins, i_sin.ins, sync=False, reason="act table order")

            # ---- GpSimd: independent tensor_tensor/tensor_scalar work ----
            FD1 = tl("FD1")  # x1 - x2
            nc.gpsimd.tensor_tensor(FD1[:], X1[:], X2[:], Alu.subtract)
            FD2 = tl("FD2")  # v1 - v2
            nc.gpsimd.tensor_tensor(FD2[:], V1[:], V2[:], Alu.subtract)
            MASK = pool.tile([P, F], mybir.dt.uint8, tag="MASK", name="MASK")
            nc.gpsimd.tensor_single_scalar(MASK[:], AV[:], KARNOPP_DV, Alu.is_lt)

            # ---- DVE base chains ----
            E = tl("E")      # e = 0.5 sin(0.5t) - x2 = -0.5*S - x2
            nc.vector.scalar_tensor_tensor(E[:], S[:], -0.5, X2[:], Alu.mult, Alu.subtract)
            DXC = tl("DXC")  # d_xc = -p*xc + e
            nc.vector.scalar_tensor_tensor(DXC[:], XC[:], -c["p"], E[:], Alu.mult, Alu.add)
            nc.sync.dma_start(out=o_r[4], in_=DXC[:])

            H = tl("H")      # F_net / K2 = (x1-x2) + (C2/K2)(v1-v2)
            nc.vector.scalar_tensor_tensor(H[:], FD2[:], C2 / K2, FD1[:], Alu.mult, Alu.add)

            TAU = tl("TAU")
            nc.vector.tensor_single_scalar(TAU[:], E[:], c["K"] / M1, Alu.mult)
            TAU2 = tl("TAU2")
            nc.vector.scalar_tensor_tensor(TAU2[:], XC[:], c["A"] / M1, TAU[:], Alu.mult, Alu.add)
            TAU3 = tl("TAU3")
            nc.vector.scalar_tensor_tensor(TAU3[:], X1[:], -K1 / M1, TAU2[:], Alu.mult, Alu.add)
            TAU4 = tl("TAU4")
            nc.vector.scalar_tensor_tensor(TAU4[:], V1[:], -C1 / M1, TAU3[:], Alu.mult, Alu.add)
            DV1 = tl("DV1")
            nc.vector.scalar_tensor_tensor(DV1[:], H[:], -K2 / M1, TAU4[:], Alu.mult, Alu.add)
            nc.sync.dma_start(out=o_r[1], in_=DV1[:])

            # ---- clip bounds (independent of the Horner chain) ----
            MX = tl("MX")
            nc.vector.tensor_tensor(MX[:], H[:], NLP[:], Alu.max)
            MM = tl("MM")
            nc.vector.tensor_tensor(MM[:], MX[:], LP[:], Alu.min)

            # ---- polynomial for g0 (odd part of the kinetic head) ----
            W = tl("W", hdt)
            nc.vector.tensor_scalar(W[:], U[:], c["su"], c["bu"], Alu.mult, Alu.add)
            acc = tl("ACCa", hdt)
            nc.vector.tensor_single_scalar(acc[:], W[:], oc[deg], Alu.mult)
            flip = False
            for k in range(deg - 1, 0, -1):
                nxt = tl("ACCb" if not flip else "ACCa", hdt)
                nc.vector.scalar_tensor_tensor(nxt[:], acc[:], oc[k], W[:], Alu.add, Alu.mult)
                acc = nxt
                flip = not flip
            G0 = tl("G0", hdt)
            nc.vector.scalar_tensor_tensor(G0[:], acc[:], oc[0], V2H[:], Alu.add, Alu.mult)

            # ---- ACT phase 2 (table set: natural_log_exp_and_others) ----
            Q = tl("Q")
            nc.scalar.activation(Q[:], G0[:], Act.Exp, bias=c["exp_bias"])
            KIN = tl("KIN")  # softplus(g0 + b2[0]) = ln(1 + exp(...))
            nc.scalar.activation(KIN[:], Q[:], Act.Ln, bias=1.0)

            # ---- friction select + dv2 ----
            PHI = tl("PHI")  # kinetic/K2 * sign(v2) = -F_kinetic/K2
            nc.vector.scalar_tensor_tensor(PHI[:], KIN[:], 1.0 / K2, SGN[:], Alu.mult, Alu.mult)
            # PHI <- where(|v2| < dv, MM, PHI): now PHI == -F_friction/K2
            nc.vector.copy_predicated(PHI[:], MASK[:], MM[:])
            D1 = tl("D1")    # h - PHI = (F_net + F_friction)/K2
            nc.vector.scalar_tensor_tensor(D1[:], PHI[:], -1.0, H[:], Alu.mult, Alu.add)
            DV2 = tl("DV2")
            nc.scalar.activation(DV2[:], D1[:], Act.Identity, scale=K2 / M2)
            nc.sync.dma_start(out=o_r[3], in_=DV2[:])

    nc.finalize()
    return nc


def _prepare(inputs):
    """Host-side constant folding + program build (cached on weight values)."""
    logK = np.float32(inputs["logK"]); logz = np.float32(inputs["logz"])
    logp = np.float32(inputs["logp"])
    W1 = np.asarray(inputs["W1"], dtype=np.float32)
    b1 = np.asarray(inputs["b1"], dtype=np.float32)
    W2 = np.asarray(inputs["W2"], dtype=np.float32)
    b2 = np.asarray(inputs["b2"], dtype=np.float32)
    v2 = np.asarray(inputs["z"][3], dtype=np.float32)
    vmax = float(np.abs(v2).max()) * 1.02 + 1e-3

    key = (logK.tobytes(), logz.tobytes(), logp.tobytes(), W1.tobytes(),
           b1.tobytes(), W2.tobytes(), b2.tobytes(), round(vmax, 3))
    if key in _compile_cache:
        return _compile_cache[key]

    K = np.float32(np.exp(logK))
    z_ctrl = np.float32(np.exp(logz))
    p_ctrl = np.float32(np.exp(logp))
    A = np.float32(K * (z_ctrl - p_ctrl))

    fit = _fit_friction(W1, b1, W2, b2, vmax)

    consts = dict(
        K=float(K), p=float(p_ctrl), A=float(A),
        su=fit["su"], bu=fit["bu"], exp_bias=fit["exp_bias"],
        L0_d=fit["L0"] / K2, L1_d=fit["L1"] / K2,
        oc=fit["oc"],
    )
    nc = _build_program(consts)
    _compile_cache[key] = (nc, fit)
    return nc, fit


def _run(inputs, trace=False):
    from concourse.bass_utils import run_bass_kernel_spmd

    nc, _fit = _prepare(inputs)

    t = np.ascontiguousarray(np.asarray(inputs["t"], dtype=np.float32))
    z = np.ascontiguousarray(np.asarray(inputs["z"], dtype=np.float32))
    in_maps = []
    for i in range(N_CORES):
        sl = slice(i * N_CORE, (i + 1) * N_CORE)
        in_maps.append({"t": np.ascontiguousarray(t[sl]),
                        "z": np.ascontiguousarray(z[:, sl])})

    res = run_bass_kernel_spmd(nc, in_maps, core_ids=list(range(N_CORES)),
                               trace=trace)
    out = np.empty((5, N_TOTAL), dtype=np.float32)
    for i in range(N_CORES):
        out[:, i * N_CORE:(i + 1) * N_CORE] = res.results[i]["out"]
    return out, res


def kernel(**inputs):
    out, _res = _run(inputs, trace=False)
    return out


# revision 12
# speedup vs baseline: 1.2716x; 1.2716x over previous
"""Trainium2 Bass kernel: batched controlled-system dynamics (N = 2^20 states).

Strategy:
  - Pure data parallel over 8 NeuronCores: contiguous slices of the batch axis.
  - Per core the workload is one [128, 1024] fp32 SBUF "ribbon" per vector.
  - The friction MLP (1 -> 64 -> 2, tanh + softplus heads) depends only on the
    scalar v2, so it is collapsed host-side into 1D functions of v2:
      kinetic(v2)  = softplus(g0(v2) + b2[0]),  g0 odd for b1 == 0
      stiction(v2) = softplus(g1(v2) + b2[1]),  only needed for |v2| < 0.01
    g0 is approximated as v2 * O(w), O a polynomial in w = affine(v2^2), fit at
    runtime from the actual W1/W2/b1/b2; softplus is evaluated on-chip as
    Ln(1 + Exp(x)); stiction is linearized around 0 (exact to ~1e-4 on the
    +-0.01 window where it is used).
  - Work is split across engines: Vector (fused scalar_tensor_tensor chains +
    Horner), GpSimd (independent tensor_tensor/tensor_scalar ops), Scalar
    (Sin/Square/Sign/Abs + Exp/Ln + Identity-affine; 2 activation-table sets).
"""

import numpy as np

# physical system constants (match the reference)
M1, M2 = 1.0, 1.5
K1, K2 = 2.0, 3.0
C1, C2 = 0.5, 0.8
KARNOPP_DV = 0.01
REF_AMP, REF_OMEGA = 0.5, 0.5

N_CORES = 8
N_TOTAL = 1 << 20
N_CORE = N_TOTAL // N_CORES    # 131072
P = 128
F = N_CORE // P                # 1024

HORNER_BF16 = False
FIT_TOL = 5e-3

_compile_cache = {}


def _softplus(x):
    return np.log1p(np.exp(-np.abs(x))) + np.maximum(x, 0.0)


def _fit_friction(W1, b1, W2, b2, vmax):
    """Fit the 1D collapse of the friction MLP (see module docstring)."""
    W1 = W1.astype(np.float64).reshape(-1)      # [H]
    b1 = b1.astype(np.float64).reshape(-1)
    W2 = W2.astype(np.float64)                  # [H, 2]
    b2 = b2.astype(np.float64).reshape(-1)

    def gg(v, col):
        return np.tanh(np.outer(v, W1) + b1) @ W2[:, col]

    umax = vmax * vmax
    su = 2.0 / umax
    bu = -1.0

    M = 4000
    wn = np.cos(np.pi * (np.arange(M) + 0.5) / M)
    u = (wn + 1.0) / 2.0 * umax
    v = np.sqrt(np.maximum(u, 1e-12))
    gp = gg(v, 0)
    gm = gg(-v, 0)
    E = (gp + gm) / 2.0          # even part of g0 (== 0 when b1 == 0)
    O = (gp - gm) / 2.0 / v      # odd part / v, a function of u (hence w)

    exp_bias = b2[0] + float(np.mean(E))

    vchk = np.linspace(KARNOPP_DV * 0.9, vmax, 30000)
    uchk = vchk * vchk
    wchk = uchk * su + bu
    g0chk = (gg(vchk, 0) - gg(-vchk, 0)) / 2.0

    weight = v + 0.02
    import numpy.polynomial.chebyshev as C
    import numpy.polynomial.polynomial as Pp

    best = None
    for deg in range(6, 17):
        cc = C.chebfit(wn, O, deg, w=weight)
        mono = C.cheb2poly(cc)
        err = np.abs(vchk * Pp.polyval(wchk, mono) - g0chk).max()
        best = (mono, err)
        if err < FIT_TOL:
            break
    mono, err = best

    # stiction limit, linearized at 0:  L(v) = softplus(g1(v) + b2[1])
    d = 1e-4
    g1p = (gg(np.array([d]), 1)[0] - gg(np.array([-d]), 1)[0]) / (2 * d)
    g10 = gg(np.array([0.0]), 1)[0] + b2[1]
    L0 = _softplus(g10)
    sig = 1.0 / (1.0 + np.exp(-g10))
    L1 = sig * g1p

    return dict(oc=mono, su=su, bu=bu, exp_bias=exp_bias, L0=L0, L1=L1,
                fit_err=err, deg=len(mono) - 1)


def _build_program(consts):
    """Build the SPMD Bass program (same on all 8 cores)."""
    import concourse.bacc as bacc
    import concourse.mybir as mybir
    from concourse import tile
    from concourse.tile_rust import add_dep_helper

    fp32 = mybir.dt.float32
    bf16 = mybir.dt.bfloat16
    hdt = bf16 if HORNER_BF16 else fp32
    Alu = mybir.AluOpType
    Act = mybir.ActivationFunctionType

    c = {k: float(np.float32(v)) for k, v in consts.items() if np.isscalar(v)}
    oc = [float(np.float32(x)) for x in consts["oc"]]
    deg = len(oc) - 1

    nc = bacc.Bacc()

    # activation float biases lower to [128,1] const SBUF APs; register the
    # ones this program uses (0.0 / 1.0 ship with Bass already)
    def reg_const(val):
        v = float(val)
        if (fp32, v) not in nc.const_aps.aps:
            tsr = nc.alloc_sbuf_tensor(f"constu-f32-{len(nc.const_aps.aps)}", [128, 1], fp32)
            nc.gpsimd.memset(tsr.ap(), v)
            nc.const_aps.aps[(fp32, v)] = tsr.ap()

    neg_pi = float(np.float32(-np.pi))
    for v in (neg_pi, c["exp_bias"], c["L0_d"], -c["L0_d"]):
        reg_const(v)
    nc.all_engine_barrier()

    t_d = nc.dram_tensor("t", [N_CORE], fp32, kind="ExternalInput")
    z_d = nc.dram_tensor("z", [5, N_CORE], fp32, kind="ExternalInput")
    out_d = nc.dram_tensor("out", [5, N_CORE], fp32, kind="ExternalOutput")

    t_r = t_d[:].rearrange("(p f) -> p f", p=P)
    z_r = [z_d[i, :].rearrange("(p f) -> p f", p=P) for i in range(5)]
    o_r = [out_d[i, :].rearrange("(p f) -> p f", p=P) for i in range(5)]

    with tile.TileContext(nc) as tc:
        with tc.tile_pool(name="sb", bufs=1) as pool:
            def tl(tag, dt=fp32):
                return pool.tile([P, F], dt, tag=tag, name=tag)

            T = tl("T"); X1 = tl("X1"); V1 = tl("V1"); X2 = tl("X2")
            V2 = tl("V2"); XC = tl("XC")

            # loads
            nc.sync.dma_start(out=T[:], in_=t_r)
            nc.sync.dma_start(out=X1[:], in_=z_r[0])
            nc.sync.dma_start(out=V1[:], in_=z_r[1])
            nc.sync.dma_start(out=X2[:], in_=z_r[2])
            nc.sync.dma_start(out=V2[:], in_=z_r[3])
            nc.sync.dma_start(out=XC[:], in_=z_r[4])

            # passthrough rows: dx1 = v1, dx2 = v2
            nc.sync.dma_start(out=o_r[0], in_=V1[:])
            nc.sync.dma_start(out=o_r[2], in_=V2[:])

            # ---- ACT phase 1 (table set: trig_and_small) ----
            S = tl("S")      # S = sin(0.5 t - pi) = -sin(0.5 t)
            i_sin = nc.scalar.activation(S[:], T[:], Act.Sin, bias=neg_pi, scale=0.5)
            U = tl("U")      # v2^2
            i_u = nc.scalar.activation(U[:], V2[:], Act.Square)
            SGN = tl("SGN")
            i_sgn = nc.scalar.activation(SGN[:], V2[:], Act.Sign)
            AV = tl("AV")
            i_av = nc.scalar.activation(AV[:], V2[:], Act.Abs)
            # stiction limit (scaled by 1/K2), linear in v2 — Identity is in
            # every table set so these never force a table switch
            LP = tl("LP"); NLP = tl("NLP")
            i_lp = nc.scalar.activation(LP[:], V2[:], Act.Identity,
                                        bias=c["L0_d"], scale=c["L1_d"])
            i_nlp = nc.scalar.activation(NLP[:], V2[:], Act.Identity,
                                         bias=-c["L0_d"], scale=-c["L1_d"])
            # keep every phase-1 ACT op after Sin so exactly one trig load +
            # one exp/ln load are inserted (Identity/Square/... are in every
            # set; ordering just stops a third "default" set load)
            for bi in (i_u, i_sgn, i_av, i_lp, i_nlp):
                add_dep_helper(bi.ins, i_sin.ins, sync=False, reason="act table order")

            FD1 = tl("FD1")  # x1 - x2
            nc.vector.tensor_tensor(FD1[:], X1[:], X2[:], Alu.subtract)
            FD2 = tl("FD2")  # v1 - v2
            nc.vector.tensor_tensor(FD2[:], V1[:], V2[:], Alu.subtract)
            MASK = pool.tile([P, F], mybir.dt.uint8, tag="MASK", name="MASK")
            nc.vector.tensor_single_scalar(MASK[:], AV[:], KARNOPP_DV, Alu.is_lt)

            # ---- DVE base chains ----
            E = tl("E")      # e = 0.5 sin(0.5t) - x2 = -0.5*S - x2
            nc.vector.scalar_tensor_tensor(E[:], S[:], -0.5, X2[:], Alu.mult, Alu.subtract)
            DXC = tl("DXC")  # d_xc = -p*xc + e
            nc.vector.scalar_tensor_tensor(DXC[:], XC[:], -c["p"], E[:], Alu.mult, Alu.add)
            nc.sync.dma_start(out=o_r[4], in_=DXC[:])

            H = tl("H")      # F_net / K2 = (x1-x2) + (C2/K2)(v1-v2)
            nc.vector.scalar_tensor_tensor(H[:], FD2[:], C2 / K2, FD1[:], Alu.mult, Alu.add)

            TAU = tl("TAU")
            nc.vector.tensor_single_scalar(TAU[:], E[:], c["K"] / M1, Alu.mult)
            TAU2 = tl("TAU2")
            nc.vector.scalar_tensor_tensor(TAU2[:], XC[:], c["A"] / M1, TAU[:], Alu.mult, Alu.add)
            TAU3 = tl("TAU3")
            nc.vector.scalar_tensor_tensor(TAU3[:], X1[:], -K1 / M1, TAU2[:], Alu.mult, Alu.add)
            TAU4 = tl("TAU4")
            nc.vector.scalar_tensor_tensor(TAU4[:], V1[:], -C1 / M1, TAU3[:], Alu.mult, Alu.add)
            DV1 = tl("DV1")
            nc.vector.scalar_tensor_tensor(DV1[:], H[:], -K2 / M1, TAU4[:], Alu.mult, Alu.add)
            nc.sync.dma_start(out=o_r[1], in_=DV1[:])

            # ---- clip bounds (independent of the Horner chain) ----
            MX = tl("MX")
            nc.vector.tensor_tensor(MX[:], H[:], NLP[:], Alu.max)
            MM = tl("MM")
            nc.vector.tensor_tensor(MM[:], MX[:], LP[:], Alu.min)

            # ---- polynomial for g0 (odd part of the kinetic head) ----
            W = tl("W", hdt)
            nc.vector.tensor_scalar(W[:], U[:], c["su"], c["bu"], Alu.mult, Alu.add)
            acc = tl("ACCa", hdt)
            nc.vector.tensor_single_scalar(acc[:], W[:], oc[deg], Alu.mult)
            flip = False
            for k in range(deg - 1, 0, -1):
                nxt = tl("ACCb" if not flip else "ACCa", hdt)
                nc.vector.scalar_tensor_tensor(nxt[:], acc[:], oc[k], W[:], Alu.add, Alu.mult)
                acc = nxt
                flip = not flip
            G0 = tl("G0", hdt)
            nc.vector.scalar_tensor_tensor(G0[:], acc[:], oc[0], V2[:], Alu.add, Alu.mult)

            # ---- ACT phase 2 (table set: natural_log_exp_and_others) ----
            Q = tl("Q")
            nc.scalar.activation(Q[:], G0[:], Act.Exp, bias=c["exp_bias"])
            KIN = tl("KIN")  # softplus(g0 + b2[0]) = ln(1 + exp(...))
            nc.scalar.activation(KIN[:], Q[:], Act.Ln, bias=1.0)

            # ---- friction select + dv2 ----
            PHI = tl("PHI")  # kinetic/K2 * sign(v2) = -F_kinetic/K2
            nc.vector.scalar_tensor_tensor(PHI[:], KIN[:], 1.0 / K2, SGN[:], Alu.mult, Alu.mult)
            # PHI <- where(|v2| < dv, MM, PHI): now PHI == -F_friction/K2
            nc.vector.copy_predicated(PHI[:], MASK[:], MM[:])
            D1 = tl("D1")    # h - PHI = (F_net + F_friction)/K2
            nc.vector.tensor_tensor(D1[:], H[:], PHI[:], Alu.subtract)
            DV2 = tl("DV2")
            nc.scalar.activation(DV2[:], D1[:], Act.Identity, scale=K2 / M2)
            nc.sync.dma_start(out=o_r[3], in_=DV2[:])

    nc.finalize()
    return nc


def _prepare(inputs):
    """Host-side constant folding + program build (cached on weight values)."""
    logK = np.float32(inputs["logK"]); logz = np.float32(inputs["logz"])
    logp = np.float32(inputs["logp"])
    W1 = np.asarray(inputs["W1"], dtype=np.float32)
    b1 = np.asarray(inputs["b1"], dtype=np.float32)
    W2 = np.asarray(inputs["W2"], dtype=np.float32)
    b2 = np.asarray(inputs["b2"], dtype=np.float32)
    v2 = np.asarray(inputs["z"][3], dtype=np.float32)
    vmax = float(np.abs(v2).max()) * 1.02 + 1e-3

    key = (logK.tobytes(), logz.tobytes(), logp.tobytes(), W1.tobytes(),
           b1.tobytes(), W2.tobytes(), b2.tobytes(), round(vmax, 3))
    if key in _compile_cache:
        return _compile_cache[key]

    K = np.float32(np.exp(logK))
    z_ctrl = np.float32(np.exp(logz))
    p_ctrl = np.float32(np.exp(logp))
    A = np.float32(K * (z_ctrl - p_ctrl))

    fit = _fit_friction(W1, b1, W2, b2, vmax)

    consts = dict(
        K=float(K), p=float(p_ctrl), A=float(A),
        su=fit["su"], bu=fit["bu"], exp_bias=fit["exp_bias"],
        L0_d=fit["L0"] / K2, L1_d=fit["L1"] / K2,
        oc=fit["oc"],
    )
    nc = _build_program(consts)
    _compile_cache[key] = (nc, fit)
    return nc, fit


def _run(inputs, trace=False):
    from concourse.bass_utils import run_bass_kernel_spmd

    nc, _fit = _prepare(inputs)

    t = np.ascontiguousarray(np.asarray(inputs["t"], dtype=np.float32))
    z = np.ascontiguousarray(np.asarray(inputs["z"], dtype=np.float32))
    in_maps = []
    for i in range(N_CORES):
        sl = slice(i * N_CORE, (i + 1) * N_CORE)
        in_maps.append({"t": np.ascontiguousarray(t[sl]),
                        "z": np.ascontiguousarray(z[:, sl])})

    res = run_bass_kernel_spmd(nc, in_maps, core_ids=list(range(N_CORES)),
                               trace=trace)
    out = np.empty((5, N_TOTAL), dtype=np.float32)
    for i in range(N_CORES):
        out[:, i * N_CORE:(i + 1) * N_CORE] = res.results[i]["out"]
    return out, res


def kernel(**inputs):
    out, _res = _run(inputs, trace=False)
    return out
